# revision 31
# baseline (speedup 1.0000x reference)
"""Sparse-attention kernel for 8 trn2 NeuronCores (Bass/Tile).

Math (reference):
    Q = x1 @ Wq.T + bq                       [N1, DIM]
    K = x2 @ Wk.T + bk                       [N2, DIM]
    scores = (Q @ K.T) / sqrt(ITEM)          [N1, N2]
    e = exp(scores) * label_map
    att = e / (sum_j e + 1e-8) * (sum_j label_map / topk + 1e-8)
    out = att @ x2                           [N1, ITEM]

Key transformations:
  * Rows of x1/label_map sharded across 8 cores (512 rows each); bk drops
    out of the normalization (it scales numerator and denominator equally).
  * Each core projects only its own 512-column shard of K.T; the shard is
    AllGathered in TWO d-chunks (4/4), each launched as soon as its d-range
    of the K projection finishes.  Two ops (not more) because every
    collective inherits the cross-core dispatch skew once at its rendezvous;
    the stores/triggers are placed so no bulk load ever delays them, and the
    gather window (~75-160us) is kept clear of bulk HBM traffic so the CC
    stream gets full bandwidth.
  * Scores are computed TRANSPOSED (eT tiles [n2-rows, own-rows]) by
    swapping matmul operands: weights = K.T chunks, moving = Q.T rows.
    The exp output lands directly in the spmm operand layout.
  * label masking is folded into the exp argument: host ships
    M = (label-1)*30 as fp8e4 (both values exact) and the kernel computes
    e = exp(scores + M), so masked entries underflow to ~e-26.
  * interactions = rowsum(label) is computed on the HOST (it is pure input
    preprocessing) and shipped as a tiny [128, IC] tensor, removing a 4MB
    label reload and the vector-engine reduction chain.
  * Row sums of e (the softmax denominator) use a ones-weight matmul
    chain accumulated across all 32 eT tiles, emitted 2 tiles behind the
    exp pipeline; its tail plus the per-row scale chain (PE transposes,
    reciprocal) are emitted AFTER the first spmm chain so the PE never
    idles across the phase boundary.
  * DMA slabs are emitted in exact consumption order at fine granularity:
    the first K-proj matmul is gated by only ~390KB, and every weight
    stream is placed so no projection chain ever waits on its weights.
  * Matmul operands are bf16 (fp32 PSUM accumulation); everything is
    host-rearranged partition-major so slabs load as contiguous multi-KB
    per-partition lines.
"""

import math

import numpy as np

try:
    import concourse.bass as bass
except ImportError:  # fresh interpreter without the boot path
    import sys

    sys.path.insert(0, "/opt/trn_rl_repo")
    import concourse.bass as bass

import ml_dtypes
import concourse.mybir as mybir
import concourse.tile as tile
from concourse import bacc, bass_isa
from concourse.bass_utils import run_bass_kernel_spmd

NCORES = 8
F32 = mybir.dt.float32
BF16 = mybir.dt.bfloat16
F8E4 = mybir.dt.float8e4
NPBF16 = ml_dtypes.bfloat16
NPF8E4 = ml_dtypes.float8_e4m3fn


def _build(S, N2, ITEM, DIMP, denom, topk_f):
    """Build the per-core Bass program.

    S     - x1 rows per core (multiple of 128)
    N2    - x2 rows (multiple of 512)
    ITEM  - feature dim (multiple of 512)
    DIMP  - projection dim padded to a multiple of 128
    denom - sqrt(original ITEM)
    """
    IC = S // 128  # own-row chunks
    JC = N2 // 128  # x2-row chunks (spmm contraction, eT partition blocks)
    JN = N2 // 512  # 512-wide blocks of x2 rows (one per core's K shard)
    TC = ITEM // 128  # feature chunks (projection contraction)
    TN = ITEM // 512  # 512-wide tiles of the output free dim
    DC = DIMP // 128  # projection-dim chunks
    assert JN == NCORES and S == 512
    Exp = mybir.ActivationFunctionType.Exp
    Mult = mybir.AluOpType.mult
    Add = mybir.AluOpType.add

    nc = bacc.Bacc("TRN2", target_bir_lowering=False, debug=False, num_devices=NCORES)
    x1t = nc.dram_tensor("x1t", [128, TC, S], BF16, kind="ExternalInput")
    wqt = nc.dram_tensor("wqt", [DC, 128, TC, 128], BF16, kind="ExternalInput")
    wkt = nc.dram_tensor("wkt", [DC, 128, TC, 128], BF16, kind="ExternalInput")
    x2m = nc.dram_tensor("x2m", [128, TC, 512], BF16, kind="ExternalInput")
    x2n = nc.dram_tensor("x2n", [TN, 128, JC, 512], BF16, kind="ExternalInput")
    lmt = nc.dram_tensor("lmt", [128, JC, S], F8E4, kind="ExternalInput")
    bq2 = nc.dram_tensor("bq2", [128, DC], F32, kind="ExternalInput")
    ia2 = nc.dram_tensor("ia2", [128, IC], F32, kind="ExternalInput")
    y = nc.dram_tensor("y", [S, ITEM], F32, kind="ExternalOutput")

    with tile.TileContext(nc) as tc:
        with (
            tc.tile_pool(name="big", bufs=1) as big,
            tc.tile_pool(name="persist", bufs=1) as persist,
            tc.tile_pool(name="s8k", bufs=16) as s8k,
            tc.tile_pool(name="lmtp", bufs=4) as lmtp,
            tc.tile_pool(name="outp", bufs=4) as outp,
            tc.tile_pool(name="dram", bufs=1, space="DRAM") as drampool,
            tc.tile_pool(name="acc", bufs=8, space="PSUM") as accp,
        ):
            bqs = persist.tile([128, DC], F32, tag="bqs")
            nc.sync.dma_start(bqs[:], bq2[:])
            ia = persist.tile([128, IC, 1], F32, tag="ia")
            nc.sync.dma_start(ia[:], ia2[:])

            # ---- phase A: own K.T shard projection, chunked AllGather ----
            # DMA queue plan (only sync/scalar/gpsimd can issue; the sync
            # queue measures ~2-3x slower than scalar/gpsimd under load, so
            # it only carries small or late traffic):
            #   gpsimd: wk0/xm interleaved in consumption order, wk1-3,
            #           ktin0 (lands exactly when the queue drains to it),
            #           x1 q0-3, wq evens | kt share | x2c share
            #   scalar: xm share, wk6 wk7, x1 q4-7, wq odds | kt | x2c
            #   sync:   bqs ia wk4 wk5 | ktin1 ktin2 | lmt | y
            # Collectives trigger from gpsimd (the only engine with the CC
            # doorbell), emitted AFTER all its bulk DMA issues so the gather
            # waits never delay descriptor issue.
            wk = []
            for d in range(DC):
                wk.append(s8k.tile([128, TC, 128], BF16, tag="s8", name=f"wk_{d}"))
            xm = []
            for q in range(4):
                xm.append(s8k.tile([128, 8, 512], BF16, tag="s8", name=f"x2m_{q}"))

            def xm_piece(eng, p):
                q, lo = p // 4, (p % 4) * 2
                eng.dma_start(
                    xm[q][:, lo : lo + 2, :], x2m[:, q * 8 + lo : q * 8 + lo + 2, :]
                )

            # chain 0 consumes (wk0[t], xm[t]) in t order at ~250GB/s; feed it
            # from both fast queues with STRICTLY t-ascending order per queue
            # (sync is too slow/starved to carry anything early-chain-gating).
            # The very first matmul gates on just wk0[t0:2] + xm[t0] (~190KB).
            nc.gpsimd.dma_start(wk[0][:, 0:2, :], wkt[0, :, 0:2, :])
            nc.gpsimd.dma_start(xm[0][:, 0:1, :], x2m[:, 0:1, :])
            nc.scalar.dma_start(xm[0][:, 1:2, :], x2m[:, 1:2, :])
            nc.gpsimd.dma_start(wk[0][:, 2:8, :], wkt[0, :, 2:8, :])
            for p in range(1, 14):
                if p % 4 == 0:
                    a = p // 4
                    nc.gpsimd.dma_start(
                        wk[0][:, a * 8 : (a + 1) * 8, :],
                        wkt[0, :, a * 8 : (a + 1) * 8, :],
                    )
                xm_piece(nc.scalar if p % 2 else nc.gpsimd, p)
            # the tail pieces (needed last, ~30us in) are within reach of the
            # slow sync queue — frees the fast queues for wk1-3
            xm_piece(nc.sync, 14)
            xm_piece(nc.sync, 15)
            for d in (1, 2, 3):
                nc.gpsimd.dma_start(wk[d][:], wkt[d])
            for d in (4, 5, 6, 7):
                nc.scalar.dma_start(wk[d][:], wkt[d])
            x1s = big.tile([128, TC, S], BF16, tag="bigA", name="x1s")
            wq = []
            for d in range(DC):
                wq.append(s8k.tile([128, TC, 128], BF16, tag="s8", name=f"wq_{d}"))

            ktsb = persist.tile([128, DC, 512], BF16, tag="ktsb")
            # 2-way chunked AllGather: launch after d2 / d7 chains finish.
            # Fewer ops = fewer cross-core rendezvous (each inherits the
            # core-dispatch skew once); the small first chunk pulls the
            # first trigger as close to the startup barrier as possible.
            CCS = [(0, 3), (3, 8)]
            ktin = [
                drampool.tile(
                    [128, hi - lo, 512], BF16, tag=f"ktin{h}", name=f"ktin{h}"
                )
                for h, (lo, hi) in enumerate(CCS)
            ]
            ktall = [
                drampool.tile(
                    [NCORES, 128, hi - lo, 512], BF16, tag=f"ktall{h}",
                    name=f"ktall{h}", addr_space="Shared",
                )
                for h, (lo, hi) in enumerate(CCS)
            ]
            for d in range(DC):
                ps = accp.tile([128, 512], F32, tag="acc", name=f"psk_{d}")
                for t in range(TC):
                    nc.tensor.matmul(
                        ps[:],
                        wk[d][:, t, :],
                        xm[t // 8][:, t % 8, :],
                        start=(t == 0),
                        stop=(t == TC - 1),
                    )
                nc.vector.tensor_copy(ktsb[:, d, :], ps[:])
                # ktin0 rides gpsimd: its queue drains to it right as the d=2
                # chain finishes (nothing else queues on gpsimd before it, so
                # the store starts the moment the data is ready).
                if d == 2:
                    nc.gpsimd.dma_start(ktin[0][:], ktsb[:, 0:3, :])
            # Everything phase-1 must be IN FLIGHT before the gathers start:
            # the collective stream needs HBM headroom from ~80us on, so x1
            # and most wq chunks front-load on scalar/gpsimd; wq5/wq7 trickle
            # on sync during the gather window (plenty of slack before their
            # chains run).
            for q in range(8):
                nc.scalar.dma_start(
                    x1s[:, q * 4 : (q + 1) * 4, :], x1t[:, q * 4 : (q + 1) * 4, :]
                )
            for d in (4, 6):
                nc.gpsimd.dma_start(wq[d][:], wqt[d])
            for d in (0, 1, 2, 3):
                nc.scalar.dma_start(wq[d][:], wqt[d])
            for d in (5, 7):
                nc.sync.dma_start(wq[d][:], wqt[d])
            # ktin1 emitted after the sync wq issues so its d=7 data-gate
            # can't head-of-line-block them; h1's start is stream-serialized
            # behind h0 anyway, so a slightly later store costs nothing.
            nc.sync.dma_start(ktin[1][:], ktsb[:, 3:8, :])
            for h in range(len(CCS)):
                nc.gpsimd.collective_compute(
                    "AllGather",
                    mybir.AluOpType.bypass,
                    replica_groups=[list(range(NCORES))],
                    ins=[ktin[h][:].opt()],
                    outs=[ktall[h][:].opt()],
                )

            # ---- phase 1: QT[d, i] = ((x1 @ Wq.T) + bq) / denom, DIM-major ----
            qt = persist.tile([128, DC, S], BF16, tag="qt")
            for d in range(DC):
                ps = accp.tile([128, 512], F32, tag="acc", name=f"psq_{d}")
                for t in range(TC):
                    nc.tensor.matmul(
                        ps[:],
                        wq[d][:, t, :],
                        x1s[:, t, :],
                        start=(t == 0),
                        stop=(t == TC - 1),
                    )
                nc.vector.tensor_scalar(
                    qt[:, d, :], ps[:], 1.0 / denom, bqs[:, d : d + 1],
                    op0=Mult, op1=Add,
                )

            # ---- phase-3 K.T tiles: preload all shards from the gathered
            # DRAM copy while the Q projection still owns the PE ----
            # h-chunk-major load order: all h0 pieces (available first), then
            # h1, then h2 — a piece gated on a later gather chunk never
            # head-of-line-blocks pieces whose data already arrived.
            kts = [
                s8k.tile([128, DC, 512], BF16, tag="s8", name=f"kt_{jn}")
                for jn in range(JN)
            ]
            for h, (lo, hi) in enumerate(CCS):
                for jn in range(JN):
                    eng = nc.scalar if (h * JN + jn) % 2 == 0 else nc.gpsimd
                    eng.dma_start(kts[jn][:, lo:hi, :], ktall[h][jn])
            # label-mask tiles (fp8, one per jn) ride the idle sync queue
            lms = []
            for jn in range(JN):
                lmc = lmtp.tile([128, 4, 512], F8E4, tag="lmt", name=f"lm_{jn}")
                nc.sync.dma_start(lmc[:], lmt[:, jn * 4 : (jn + 1) * 4, :])
                lms.append(lmc)

            # ---- phase 3: transposed scores -> exp -> eT tiles.  The e
            # row-sums accumulate on the vector engine (one add per tile,
            # right after its exp) — no PE cycles spent on them. ----
            et = big.tile([128, JC, S], BF16, tag="bigA", name="et")
            eacc = persist.tile([128, 512], F32, tag="eacc")
            for jn in range(JN):
                kt = kts[jn]
                for jl in range(4):
                    jj = jn * 4 + jl
                    ps = accp.tile([128, 512], F32, tag="acc", name=f"ps3_{jj}")
                    for d in range(DC):
                        nc.tensor.matmul(
                            ps[:],
                            kt[:, d, jl * 128 : (jl + 1) * 128],
                            qt[:, d, :],
                            start=(d == 0),
                            stop=(d == DC - 1),
                        )
                    nc.vector.tensor_add(ps[:], ps[:], lms[jn][:, jl, :])
                    nc.scalar.activation(et[:, jj, :], ps[:], Exp)
                    if jj == 0:
                        nc.vector.tensor_copy(eacc[:], et[:, 0, :])
                    else:
                        nc.vector.tensor_add(eacc[:], eacc[:], et[:, jj, :])

            # ---- softmax scale: partition-reduce the e-sums on gpsimd, then
            # a 2KB DRAM round-trip turns the row vector into per-partition
            # column layout.  Zero PE instructions — the PE rolls straight
            # from the scores chains into the spmm; the first drains just
            # wait for asb. ----
            erep = persist.tile([128, 512], F32, tag="erep")
            nc.gpsimd.partition_all_reduce(
                erep[:], eacc[:], 128, bass_isa.ReduceOp.add
            )
            edram = drampool.tile([IC, 128], F32, tag="edram", name="edram")
            nc.sync.dma_start(edram[:], erep[0:1, :])
            ecol = persist.tile([128, IC], F32, tag="ecol")
            nc.sync.dma_start(ecol[:], edram[:].rearrange("a b -> b a"))
            rec = persist.tile([128, IC, 1], F32, tag="rec")
            nc.vector.tensor_scalar_add(ecol[:], ecol[:], 1e-8)
            nc.vector.reciprocal(rec[:], ecol[:])
            asb = persist.tile([128, IC, 1], F32, tag="asb")
            nc.vector.tensor_mul(asb[:], ia[:], rec[:])

            # ---- phase 4: spmm, one 32-matmul chain per output tile ----
            for n in range(TN):
                ch = []
                for jq in range(4):
                    c = s8k.tile([128, 8, 512], BF16, tag="s8", name=f"x2c_{n}_{jq}")
                    eng = nc.scalar if jq % 2 == 0 else nc.gpsimd
                    eng.dma_start(c[:], x2n[n, :, jq * 8 : (jq + 1) * 8, :])
                    ch.append(c)
                for i in range(IC):
                    ps = accp.tile([128, 512], F32, tag="acc", name=f"ps4_{n}_{i}")
                    for j in range(JC):
                        nc.tensor.matmul(
                            ps[:],
                            et[:, j, i * 128 : (i + 1) * 128],
                            ch[j // 8][:, j % 8, :],
                            start=(j == 0),
                            stop=(j == JC - 1),
                        )
                    o = outp.tile([128, 512], F32, tag="o")
                    if i % 2 == 0:
                        nc.vector.tensor_scalar_mul(o[:], ps[:], asb[:, i, :])
                    else:
                        nc.scalar.mul(o[:], ps[:], asb[:, i, :])
                    # last column-block's stores split across two fast-ish
                    # queues so the final store latency doesn't sit on the
                    # kernel tail
                    if n == TN - 1:
                        nc.scalar.dma_start(
                            y[i * 128 : (i + 1) * 128, n * 512 : n * 512 + 256],
                            o[:, 0:256],
                        )
                        nc.sync.dma_start(
                            y[i * 128 : (i + 1) * 128, n * 512 + 256 : (n + 1) * 512],
                            o[:, 256:512],
                        )
                    else:
                        nc.sync.dma_start(
                            y[i * 128 : (i + 1) * 128, n * 512 : (n + 1) * 512],
                            o[:],
                        )

    nc.compile()
    return nc


def _pmajor(a, p, inner):
    """[R, C] with R = nblk*p -> [p, nblk, C] partition-major, where each
    partition's inner block is contiguous."""
    R, C = a.shape
    nblk = R // p
    return np.ascontiguousarray(a.reshape(nblk, p, C).transpose(1, 0, 2))


def _in_maps(x1, x2, label_map, Wq, bq, Wk, DIMP, S, denom, topk_f):
    ITEM = x1.shape[1]
    N2 = x2.shape[0]
    DIM = Wq.shape[0]
    DC = DIMP // 128
    JN = N2 // 512
    TN = ITEM // 512
    IC = S // 128

    wqp = np.zeros((DIMP, ITEM), NPBF16)
    wqp[:DIM] = Wq.astype(NPBF16)
    wkp = np.zeros((DIMP, ITEM), NPBF16)
    wkp[:DIM] = Wk.astype(NPBF16)
    bqp = np.zeros((DIMP,), np.float32)
    bqp[:DIM] = bq / denom
    bq2 = np.ascontiguousarray(bqp.reshape(DC, 128).T)

    x1b = x1.astype(NPBF16)
    x2b = x2.astype(NPBF16)
    wqT = np.ascontiguousarray(wqp.T)  # [ITEM, DIMP]
    x2T = np.ascontiguousarray(x2b.T)  # [ITEM, N2]

    # wqt[d] = WqT[:, d-chunk] as [128, TC, 128] partition-major
    wqt = np.stack(
        [_pmajor(wqT[:, d * 128 : (d + 1) * 128], 128, None) for d in range(DC)]
    )
    wkT = np.ascontiguousarray(wkp.T)
    wktb = np.stack(
        [_pmajor(wkT[:, d * 128 : (d + 1) * 128], 128, None) for d in range(DC)]
    )
    # x2t[jn] = x2T[:, jn-chunk] as [128, TC, 512]
    x2tb = np.stack(
        [_pmajor(x2T[:, j * 512 : (j + 1) * 512], 128, None) for j in range(JN)]
    )
    # x2n[n] = x2[:, n-chunk] as [128, JC, 512]
    x2nb = np.stack(
        [_pmajor(x2b[:, n * 512 : (n + 1) * 512], 128, None) for n in range(TN)]
    )
    maps = []
    for c in range(NCORES):
        sl = slice(c * S, (c + 1) * S)
        shard = label_map[sl]
        # transposed additive mask: 0 where label=1, -30 where label=0 (fp8)
        mt = ((shard.T.astype(np.float32) - 1.0) * 30.0).astype(NPF8E4)  # [N2, S]
        lmtb = _pmajor(mt, 128, None)  # [128, JC, S]
        # interactions scale, host-side: (rowsum(label)/topk + 1e-8), laid out
        # [partition, chunk] so row r -> (r % 128, r // 128)
        iav = (shard.astype(np.float32).sum(axis=1) / topk_f + 1e-8).astype(
            np.float32
        )
        ia2 = np.ascontiguousarray(iav.reshape(IC, 128).T)
        maps.append(
            {
                "x1t": _pmajor(np.ascontiguousarray(x1b[sl].T), 128, None),
                "wqt": wqt,
                "wkt": wktb,
                "x2m": x2tb[c],
                "x2n": x2nb,
                "lmt": lmtb,
                "bq2": bq2,
                "ia2": ia2,
            }
        )
    return maps


def _run(x1, x2, label_map, Wq, bq, Wk, bk, topk, trace=False):
    x1 = np.asarray(x1, np.float32)
    x2 = np.asarray(x2, np.float32)
    label_map = np.asarray(label_map, np.float32)
    Wq = np.asarray(Wq, np.float32)
    bq = np.asarray(bq, np.float32)
    Wk = np.asarray(Wk, np.float32)
    N1, ITEM = x1.shape
    N2 = x2.shape[0]
    DIM = Wq.shape[0]
    S = N1 // NCORES
    DIMP = ((DIM + 127) // 128) * 128
    denom = math.sqrt(ITEM)
    nc = _build(S, N2, ITEM, DIMP, denom, float(topk))
    maps = _in_maps(x1, x2, label_map, Wq, bq, Wk, DIMP, S, denom, float(topk))
    res = run_bass_kernel_spmd(
        nc, maps, list(range(NCORES)), trace=trace, trace_cores=[0] if trace else None
    )
    out = np.concatenate([res.results[c]["y"] for c in range(NCORES)], axis=0)
    return out.astype(np.float32), res


def kernel(x1, x2, label_map, Wq, bq, Wk, bk, topk):
    out, _ = _run(x1, x2, label_map, Wq, bq, Wk, bk, topk)
    return out
